# revision 46
# baseline (speedup 1.0000x reference)
"""Trainium2 Bass kernel for AttentionalPlanarRemapping.

out[n,c,h,w] = sum_d softmax(atts[n,c,:])[d] * images[n,d,h,w]

Per-sample: W = softmax(atts[n]) [C,C]; out[n] = W @ images[n].reshape(C, H*W).

Sharding: data-parallel over N across 8 cores (4 samples per core).

Host preprocessing inside kernel(): atts is cast to fp16 and uploaded
TRANSPOSED and PAIR-PACKED, attsT[po, d, j, c] = atts[2*po+j][c, d]: the
contraction dim d lands on partitions (the matmul lhsT layout) and every
DRAM row holds a sample pair, so rows are 2KB -- DMA packet cost is
~flat per packet, so 1KB rows would waste half the ring. images/out are
fp16 (values only -- the returned array is float32).

Per-core plan, per sample (software-pipelined: prep(n+1) is emitted
before compute(n); prep uses only the sync/scalar queues so it can never
block compute's vector-engine evictions -> PSUM slot reuse -> PE):
  prep(n):    DMA pair attsT[n//2] (even n) and images[n] on the sync
              ring; E = exp(attsT slice) fp16 (ACT, one instr; no
              max-sub: |atts| < 6 so exp is safe)
  compute(n): denominator: E2 = sum_kd E (3 DVE f16 adds at 2x rate,
              emitted during the PREVIOUS sample so they never block) ->
              ONE ones.T @ E2 matmul -> s replicated; s_sb fp16 copy;
              rp redistribution (4 tiny fp16 PE matmuls s_sb-blk.T @
              (1/128), ~free with shared weights) -> r = 1/s (DVE);
              main matmuls kc=0..3 into ps[128,1024] (PSUM pool depth
              3); evict o = ps * r[:,kc] -> fp16 (DVE kc<2, ACT kc>=2);
              store per kc on alternating SWDGE (gpsimd)/HWDGE (scalar).
Sample 0 (ramp): loads chunked per kd and interleaved, exp per chunk,
kd-OUTER matmul order over bands 0-2 so each arriving chunk feeds 6
matmuls immediately (dense PE -> earlier HAM warm flip, full transfer
overlap), denominator summed with one ones-matmul per kd chunk, band 3
on the two 1-bank s/rp PSUM slots. Last sample: all-vector evictions
(scalar is free of exp there), final band as two independent 1-bank
PSUM chains evicted in parallel (ACT + DVE) and stored on the HWDGE
rings (sync + scalar), which drain much faster at kernel end than
SWDGE. PE cold-start warm-up was tried and REVERTED: ~5us of dummy
matmul activity trips a sustained utilization limiter that slows all
samples (216 -> 259ns spacing).
"""

import numpy as np
from contextlib import ExitStack

import concourse.bass as bass
import concourse.mybir as mybir
import concourse.tile as tile
from concourse import bacc
from concourse.bass_utils import run_bass_kernel_spmd

N, C, H, W = 32, 512, 32, 32
HW = H * W                      # 1024
NCORES = 8
NPC = N // NCORES               # 4 samples per core
P = 128
KC = C // P                     # 4 chunks over output channel c
KD = C // P                     # 4 chunks over contraction d
NT = 512                        # matmul moving free dim (one PSUM bank of f32)
NHT = HW // NT                  # 2

F32 = mybir.dt.float32
F16 = mybir.dt.float16
AF = mybir.ActivationFunctionType


def build_nc():
    nc = bacc.Bacc("TRN2", target_bir_lowering=False, debug=False)

    images = nc.dram_tensor("images", [NPC, C, HW], F16, kind="ExternalInput").ap()
    # attsT packed in sample PAIRS: attsT[po, d, j, c] = atts[2*po+j][c, d].
    # 2KB DRAM rows (vs 1KB per-sample) -> half the DMA packets, and each
    # pair transfer delivers the next sample's weights for free.
    attsT = nc.dram_tensor("attsT", [NPC // 2, C, 2, C], F16, kind="ExternalInput").ap()
    out = nc.dram_tensor("out", [NPC, C, HW], F16, kind="ExternalOutput").ap()

    with ExitStack() as ctx:
        tc = ctx.enter_context(tile.TileContext(nc))

        const_pool = ctx.enter_context(tc.tile_pool(name="const", bufs=1))
        ones = const_pool.tile([P, NT], F16)
        oinv = const_pool.tile([P, 2], F16)

        a_pool = ctx.enter_context(tc.tile_pool(name="a", bufs=2))
        e_pool = ctx.enter_context(tc.tile_pool(name="e", bufs=4))
        x_pool = ctx.enter_context(tc.tile_pool(name="x", bufs=4))
        o_pool = ctx.enter_context(tc.tile_pool(name="o", bufs=8))
        st_pool = ctx.enter_context(tc.tile_pool(name="st", bufs=2))
        sm_psum = ctx.enter_context(tc.tile_pool(name="smp", bufs=1, space="PSUM"))
        mm_psum = ctx.enter_context(tc.tile_pool(name="mmp", bufs=3, space="PSUM"))

        a_tiles = {}

        def prep(n, fine=False):
            """Input DMAs + exp for sample n (sync + scalar queues only)."""
            po = n // 2
            if n % 2 == 0:
                a_tiles[po] = a_pool.tile(
                    [P, KD, 2, C], F16, name=f"a{po}", tag="a"
                )
            a_t = a_tiles[po]
            x_t = x_pool.tile([P, KD, HW], F16, name=f"x{n}", tag="x")
            e_t = e_pool.tile([P, KD, C], F16, name=f"e{n}", tag="e")
            if fine:
                nc.gpsimd.memset(ones[:], 1.0)
                nc.gpsimd.memset(oinv[:], 1.0 / P)
                # all chunks interleaved on the sync ring (~300GB/s; the
                # scalar HWDGE ring measured ~85GB/s with ~3us startup --
                # tried twice, always a regression; PE warm-up dummies also
                # tried twice and reverted: sample 0 just becomes
                # transfer-paced and the extra duty feeds the sustained
                # util limiter)
                for kd in range(KD):
                    nc.sync.dma_start(
                        a_t[:, kd], attsT[po][kd * P : (kd + 1) * P]
                    )
                    nc.sync.dma_start(x_t[:, kd], images[n][kd * P : (kd + 1) * P])
                for kd in range(KD):
                    nc.scalar.activation(
                        e_t[:, kd], a_t[:, kd, n % 2], AF.Exp, bias=0.0, scale=1.0
                    )
            else:
                if n % 2 == 0:
                    nc.sync.dma_start(
                        a_t[:], attsT[po].rearrange("(kd p) j c -> p kd j c", p=P)
                    )
                nc.sync.dma_start(
                    x_t[:], images[n].rearrange("(kd p) f -> p kd f", p=P)
                )
                nc.scalar.activation(
                    e_t[:], a_t[:, :, n % 2], AF.Exp, bias=0.0, scale=1.0
                )
            return e_t, x_t

        def emit_e2(n, e_t):
            """Pre-sum E over kd on DVE (f16, 2x rate) so the replicated
            denominator needs only ONE ones-matmul on the PE."""
            e2a = st_pool.tile([P, C], F16, name=f"e2a{n}", tag="e2a")
            nc.vector.tensor_add(e2a[:], e_t[:, 0], e_t[:, 1])
            e2b = st_pool.tile([P, C], F16, name=f"e2b{n}", tag="e2b")
            nc.vector.tensor_add(e2b[:], e_t[:, 2], e_t[:, 3])
            e2 = st_pool.tile([P, C], F16, name=f"e2{n}", tag="e2")
            nc.vector.tensor_add(e2[:], e2a[:], e2b[:])
            return e2

        def emit_ones(n, e2):
            """Replicated denominators: s_ps[p, c] = sum_d E[d, c] (PE)."""
            s_ps = sm_psum.tile([P, C], F32, name=f"s{n}", tag="s", space="PSUM")
            nc.tensor.matmul(s_ps[:], lhsT=ones[:, 0:P], rhs=e2[:])
            s_sb = st_pool.tile([P, C], F16, name=f"ssb{n}", tag="ssb")
            nc.vector.tensor_copy(s_sb[:], s_ps[:])
            return s_sb

        def emit_rp(n, s_sb, r_t):
            """Redistribute s to per-partition layout via tiny PE matmuls,
            then r = 1/s on DVE."""
            rp_ps = sm_psum.tile([P, 2 * KC], F32, name=f"rp{n}", tag="rp", space="PSUM")
            for j in range(KC):
                nc.tensor.matmul(
                    rp_ps[:, j * 2 : (j + 1) * 2],
                    lhsT=s_sb[:, j * P : (j + 1) * P],
                    rhs=oinv[:],
                )
            s_col = st_pool.tile([P, KC], F32, name=f"scol{n}", tag="scol")
            nc.vector.tensor_copy(
                s_col[:],
                rp_ps[:].rearrange("p (kc j) -> p kc j", j=2)[:, :, 0],
            )
            nc.vector.reciprocal(r_t[:], s_col[:])

        def emit_mms(kc, e_t, x_t, ps, ht_major=False):
            if ht_major:
                # ht-major: the first half's accumulation group completes
                # 4 matmuls early, so its eviction overlaps the second
                # half's matmuls (used for the very last band)
                for ht in range(NHT):
                    for kd in range(KD):
                        nc.tensor.matmul(
                            ps[:, ht * NT : (ht + 1) * NT],
                            lhsT=e_t[:, kd, kc * P : (kc + 1) * P],
                            rhs=x_t[:, kd, ht * NT : (ht + 1) * NT],
                            start=(kd == 0),
                            stop=(kd == KD - 1),
                        )
            else:
                for kd in range(KD):
                    lhs = e_t[:, kd, kc * P : (kc + 1) * P]
                    for ht in range(NHT):
                        nc.tensor.matmul(
                            ps[:, ht * NT : (ht + 1) * NT],
                            lhsT=lhs,
                            rhs=x_t[:, kd, ht * NT : (ht + 1) * NT],
                            start=(kd == 0),
                            stop=(kd == KD - 1),
                        )

        def emit_evict(n, kc, ps, r_t, engine, store=None):
            r_ap = r_t[:, kc : kc + 1]
            o_t = o_pool.tile([P, HW], F16, name=f"o{n}_{kc}", tag="o")
            dst = out[n][kc * P : (kc + 1) * P]
            if engine == "vector":
                nc.vector.tensor_scalar_mul(o_t[:], ps[:], r_ap)
            else:
                nc.scalar.mul(o_t[:], ps[:], r_ap)
            if store is None:
                store = nc.gpsimd if kc % 2 == 0 else nc.scalar
            store.dma_start(dst, o_t[:])

        def emit_last_band(n, kc, e_t, x_t, r_t):
            """Final band: two independent 1-bank PSUM tiles (reusing the
            s/rp slots, which are dead by now), so each half's matmul ->
            evict -> store chain is fully independent across engines and
            queues -- minimizes the kernel tail. Stores go on the HWDGE
            rings (scalar + sync), which drain much faster at kernel end
            than the gpsimd SWDGE ring."""
            r_ap = r_t[:, kc : kc + 1]
            dst = out[n][kc * P : (kc + 1) * P]
            ps_a = sm_psum.tile([P, NT], F32, name=f"tla{n}", tag="s", space="PSUM")
            ps_b = sm_psum.tile([P, NT], F32, name=f"tlb{n}", tag="rp", space="PSUM")
            for ht, ps_h in ((0, ps_a), (1, ps_b)):
                for kd in range(KD):
                    nc.tensor.matmul(
                        ps_h[:],
                        lhsT=e_t[:, kd, kc * P : (kc + 1) * P],
                        rhs=x_t[:, kd, ht * NT : (ht + 1) * NT],
                        start=(kd == 0),
                        stop=(kd == KD - 1),
                    )
                if ht == 0:
                    o_a = o_pool.tile([P, NT], F16, name=f"oa{n}", tag="oa")
                    nc.scalar.mul(o_a[:], ps_a[:], r_ap)
                    nc.scalar.dma_start(dst[:, 0:NT], o_a[:])
            o_b = o_pool.tile([P, NT], F16, name=f"ob{n}", tag="ob")
            nc.vector.tensor_scalar_mul(o_b[:], ps_b[:], r_ap)
            # final store split across both idle HWDGE rings: halves the
            # last transfer, whose completion latency gates kernel end
            nc.sync.dma_start(dst[:, NT : NT + NT // 2], o_b[:, 0 : NT // 2])
            nc.scalar.dma_start(dst[:, NT + NT // 2 : HW], o_b[:, NT // 2 : NT])

        # eviction engine per kc: DVE first (its queue is free of prep
        # work), ACT for the later bands (after exp(n+1) has drained)
        EV_ENGINE = ["vector", "vector", "scalar", "scalar"]

        def compute_first(n, e_t, x_t, e2, nxt_e=None):
            """Sample 0 (ramp): kd-outer matmul order over bands 0-2 so each
            arriving (attsT, images) chunk pair immediately feeds 6 matmuls
            -- the PE stays dense (flips the HAM cold window sooner) and
            fully overlaps the chunked loads. Band 3 runs on the two 1-bank
            s/rp PSUM slots so it needn't wait for band 0's eviction."""
            r_t = st_pool.tile([P, KC], F32, name=f"r{n}", tag="r")
            ps = [
                mm_psum.tile([P, HW], F32, name=f"ps{n}_{kc}", tag="ps", space="PSUM")
                for kc in range(3)
            ]
            nxt_e2 = None
            # sample 0 sums the denominator with one ones-matmul per kd
            # group (each needs only its own exp chunk -- no E2 wait).
            # (Alternative with the E2 tree + a single mid-stream ones
            # matmul measured the same within run noise.)
            s_ps = sm_psum.tile([P, C], F32, name=f"s{n}", tag="s", space="PSUM")
            for kd in range(KD):
                for kc in range(3):
                    lhs = e_t[:, kd, kc * P : (kc + 1) * P]
                    for ht in range(NHT):
                        nc.tensor.matmul(
                            ps[kc][:, ht * NT : (ht + 1) * NT],
                            lhsT=lhs,
                            rhs=x_t[:, kd, ht * NT : (ht + 1) * NT],
                            start=(kd == 0),
                            stop=(kd == KD - 1),
                        )
                nc.tensor.matmul(
                    s_ps[:],
                    lhsT=ones[:, 0:P],
                    rhs=e_t[:, kd],
                    start=(kd == 0),
                    stop=(kd == KD - 1),
                )
            s_sb = st_pool.tile([P, C], F16, name=f"ssb{n}", tag="ssb")
            nc.vector.tensor_copy(s_sb[:], s_ps[:])
            emit_rp(n, s_sb, r_t)
            if nxt_e is not None:
                # next sample's E2 adds run in the vector idle window here,
                # before this sample's evictions
                nxt_e2 = emit_e2(n + 1, nxt_e)
            # band 3 on the 1-bank pair (s/rp slots are free by now)
            kc = KC - 1
            dst = out[n][kc * P : (kc + 1) * P]
            r_ap = r_t[:, kc : kc + 1]
            ps3a = sm_psum.tile([P, NT], F32, name=f"f3a{n}", tag="s", space="PSUM")
            ps3b = sm_psum.tile([P, NT], F32, name=f"f3b{n}", tag="rp", space="PSUM")
            for ht, ps_h in ((0, ps3a), (1, ps3b)):
                for kd in range(KD):
                    nc.tensor.matmul(
                        ps_h[:],
                        lhsT=e_t[:, kd, kc * P : (kc + 1) * P],
                        rhs=x_t[:, kd, ht * NT : (ht + 1) * NT],
                        start=(kd == 0),
                        stop=(kd == KD - 1),
                    )
            emit_evict(n, 0, ps[0], r_t, "vector")
            emit_evict(n, 1, ps[1], r_t, "vector")
            emit_evict(n, 2, ps[2], r_t, "scalar")
            o_a = o_pool.tile([P, NT], F16, name=f"oa{n}", tag="oa")
            nc.scalar.mul(o_a[:], ps3a[:], r_ap)
            nc.gpsimd.dma_start(dst[:, 0:NT], o_a[:])
            o_b = o_pool.tile([P, NT], F16, name=f"ob{n}", tag="ob")
            nc.vector.tensor_scalar_mul(o_b[:], ps3b[:], r_ap)
            nc.scalar.dma_start(dst[:, NT:HW], o_b[:])
            return nxt_e2

        def compute(n, e_t, x_t, e2, last=False, nxt_e=None):
            r_t = st_pool.tile([P, KC], F32, name=f"r{n}", tag="r")
            ps_tiles = {}
            s_sb = emit_ones(n, e2)
            ps_tiles[0] = mm_psum.tile([P, HW], F32, name=f"ps{n}_0", tag="ps", space="PSUM")
            emit_mms(0, e_t, x_t, ps_tiles[0])
            emit_rp(n, s_sb, r_t)
            # last sample: no exp(n+1) on scalar, so keep kc0-2 evictions
            # on vector and reserve scalar for the final band's first half;
            # also keep its stores off the slow-draining SWDGE ring
            ev = ["vector", "vector", "vector", "vector"] if last else EV_ENGINE
            st = [nc.sync, nc.scalar, nc.sync, None] if last else [None] * KC
            emit_evict(n, 0, ps_tiles[0], r_t, ev[0], store=st[0])
            nxt_e2 = None
            for kc in range(1, KC):
                if last and kc == KC - 1:
                    emit_last_band(n, kc, e_t, x_t, r_t)
                else:
                    ps = mm_psum.tile([P, HW], F32, name=f"ps{n}_{kc}", tag="ps", space="PSUM")
                    emit_mms(kc, e_t, x_t, ps)
                    emit_evict(n, kc, ps, r_t, ev[kc], store=st[kc])
                if kc == 1 and nxt_e is not None:
                    # deferred: next sample's E2 adds go on the vector queue
                    # AFTER this sample's first evictions so they can never
                    # delay a PSUM slot release
                    nxt_e2 = emit_e2(n + 1, nxt_e)
            return nxt_e2

        # software pipeline: prep one sample ahead so the next sample's
        # exp/loads are never queued behind this sample's evictions
        e0_t, x0_t = prep(0, fine=True)
        e2_cur = None  # sample 0 uses per-kd ones accumulation instead
        staged = (e0_t, x0_t)
        for n in range(NPC):
            nxt = prep(n + 1) if n + 1 < NPC else None
            nxt_e = nxt[0] if nxt is not None else None
            if n == 0:
                e2_cur = compute_first(n, *staged, e2_cur, nxt_e=nxt_e)
            else:
                e2_cur = compute(
                    n, *staged, e2_cur, last=(n == NPC - 1), nxt_e=nxt_e
                )
            staged = nxt

    nc.compile()
    return nc


_NC_CACHE = None


def _get_nc():
    global _NC_CACHE
    if _NC_CACHE is None:
        _NC_CACHE = build_nc()
    return _NC_CACHE


def run(in_maps, **kwargs):
    """Run the SPMD kernel on cores 0..7. in_maps: one dict per core."""
    nc = _get_nc()
    return run_bass_kernel_spmd(nc, in_maps, core_ids=list(range(NCORES)), **kwargs)


def make_in_maps(images: np.ndarray, atts: np.ndarray):
    images = np.ascontiguousarray(
        np.asarray(images, dtype=np.float32).astype(np.float16)
    )
    atts = np.asarray(atts, dtype=np.float32)
    assert images.shape == (N, C, H, W), images.shape
    assert atts.shape == (N, C, C), atts.shape
    img_s = images.reshape(NCORES, NPC, C, HW)
    # pair-packed transpose: attsT[i][po, d, j, c] = atts[i*NPC+2*po+j][c, d]
    a = atts.astype(np.float16).reshape(NCORES, NPC // 2, 2, C, C)
    attsT = np.ascontiguousarray(a.transpose(0, 1, 4, 2, 3))
    return [
        {"images": np.ascontiguousarray(img_s[i]), "attsT": attsT[i]}
        for i in range(NCORES)
    ]


def kernel(images: np.ndarray, atts: np.ndarray) -> np.ndarray:
    in_maps = make_in_maps(images, atts)
    res = run(in_maps)
    outs = [res.results[i]["out"] for i in range(NCORES)]
    full = np.concatenate(outs, axis=0).reshape(N, C, H, W)
    return full.astype(np.float32)


# revision 47
# speedup vs baseline: 1.0119x; 1.0119x over previous
"""Trainium2 Bass kernel for AttentionalPlanarRemapping.

out[n,c,h,w] = sum_d softmax(atts[n,c,:])[d] * images[n,d,h,w]

Per-sample: W = softmax(atts[n]) [C,C]; out[n] = W @ images[n].reshape(C, H*W).

Sharding: data-parallel over N across 8 cores (4 samples per core).

Host preprocessing inside kernel(): atts is cast to fp16 and uploaded
TRANSPOSED and PAIR-PACKED, attsT[po, d, j, c] = atts[2*po+j][c, d]: the
contraction dim d lands on partitions (the matmul lhsT layout) and every
DRAM row holds a sample pair, so rows are 2KB -- DMA packet cost is
~flat per packet, so 1KB rows would waste half the ring. images/out are
fp16 (values only -- the returned array is float32).

Per-core plan, per sample (software-pipelined: prep(n+1) is emitted
before compute(n); prep uses only the sync/scalar queues so it can never
block compute's vector-engine evictions -> PSUM slot reuse -> PE):
  prep(n):    DMA pair attsT[n//2] (even n) and images[n] on the sync
              ring; E = exp(attsT slice) fp16 (ACT, one instr; no
              max-sub: |atts| < 6 so exp is safe)
  compute(n): denominator: E2 = sum_kd E (3 DVE f16 adds at 2x rate,
              emitted during the PREVIOUS sample so they never block) ->
              ONE ones.T @ E2 matmul -> s replicated; s_sb fp16 copy;
              rp redistribution (4 tiny fp16 PE matmuls s_sb-blk.T @
              (1/128), ~free with shared weights) -> r = 1/s (DVE);
              main matmuls kc=0..3 into ps[128,1024] (PSUM pool depth
              3); evict o = ps * r[:,kc] -> fp16 (DVE kc<2, ACT kc>=2);
              store per kc on alternating SWDGE (gpsimd)/HWDGE (scalar).
Sample 0 (ramp): loads chunked per kd and interleaved, exp per chunk,
kd-OUTER matmul order over bands 0-2 so each arriving chunk feeds 6
matmuls immediately (dense PE -> earlier HAM warm flip, full transfer
overlap), denominator summed with one ones-matmul per kd chunk, band 3
on the two 1-bank s/rp PSUM slots. Last sample: all-vector evictions
(scalar is free of exp there), final band as two independent 1-bank
PSUM chains evicted in parallel (ACT + DVE) and stored on the HWDGE
rings (sync + scalar), which drain much faster at kernel end than
SWDGE. PE cold-start warm-up was tried and REVERTED: ~5us of dummy
matmul activity trips a sustained utilization limiter that slows all
samples (216 -> 259ns spacing).
"""

import numpy as np
from contextlib import ExitStack

import concourse.bass as bass
import concourse.mybir as mybir
import concourse.tile as tile
from concourse import bacc
from concourse.bass_utils import run_bass_kernel_spmd

N, C, H, W = 32, 512, 32, 32
HW = H * W                      # 1024
NCORES = 8
NPC = N // NCORES               # 4 samples per core
P = 128
KC = C // P                     # 4 chunks over output channel c
KD = C // P                     # 4 chunks over contraction d
NT = 512                        # matmul moving free dim (one PSUM bank of f32)
NHT = HW // NT                  # 2

F32 = mybir.dt.float32
F16 = mybir.dt.float16
AF = mybir.ActivationFunctionType


def build_nc():
    nc = bacc.Bacc("TRN2", target_bir_lowering=False, debug=False)

    images = nc.dram_tensor("images", [NPC, C, HW], F16, kind="ExternalInput").ap()
    # attsT packed in sample PAIRS: attsT[po, d, j, c] = atts[2*po+j][c, d].
    # 2KB DRAM rows (vs 1KB per-sample) -> half the DMA packets, and each
    # pair transfer delivers the next sample's weights for free.
    attsT = nc.dram_tensor("attsT", [NPC // 2, C, 2, C], F16, kind="ExternalInput").ap()
    out = nc.dram_tensor("out", [NPC, C, HW], F16, kind="ExternalOutput").ap()

    with ExitStack() as ctx:
        tc = ctx.enter_context(tile.TileContext(nc))

        const_pool = ctx.enter_context(tc.tile_pool(name="const", bufs=1))
        ones = const_pool.tile([P, NT], F16)
        oinv = const_pool.tile([P, 2], F16)

        a_pool = ctx.enter_context(tc.tile_pool(name="a", bufs=3))
        e_pool = ctx.enter_context(tc.tile_pool(name="e", bufs=3))
        x_pool = ctx.enter_context(tc.tile_pool(name="x", bufs=3))
        o_pool = ctx.enter_context(tc.tile_pool(name="o", bufs=6))
        st_pool = ctx.enter_context(tc.tile_pool(name="st", bufs=2))
        sm_psum = ctx.enter_context(tc.tile_pool(name="smp", bufs=1, space="PSUM"))
        mm_psum = ctx.enter_context(tc.tile_pool(name="mmp", bufs=3, space="PSUM"))

        a_tiles = {}

        def prep(n, fine=False):
            """Input DMAs + exp for sample n (sync + scalar queues only)."""
            po = n // 2
            if n % 2 == 0:
                a_tiles[po] = a_pool.tile(
                    [P, KD, 2, C], F16, name=f"a{po}", tag="a"
                )
            a_t = a_tiles[po]
            x_t = x_pool.tile([P, KD, HW], F16, name=f"x{n}", tag="x")
            e_t = e_pool.tile([P, KD, C], F16, name=f"e{n}", tag="e")
            if fine:
                nc.gpsimd.memset(ones[:], 1.0)
                nc.gpsimd.memset(oinv[:], 1.0 / P)
                # all chunks interleaved on the sync ring (~300GB/s; the
                # scalar HWDGE ring measured ~85GB/s with ~3us startup --
                # tried twice, always a regression; PE warm-up dummies also
                # tried twice and reverted: sample 0 just becomes
                # transfer-paced and the extra duty feeds the sustained
                # util limiter)
                for kd in range(KD):
                    nc.sync.dma_start(
                        a_t[:, kd], attsT[po][kd * P : (kd + 1) * P]
                    )
                    nc.sync.dma_start(x_t[:, kd], images[n][kd * P : (kd + 1) * P])
                for kd in range(KD):
                    nc.scalar.activation(
                        e_t[:, kd], a_t[:, kd, n % 2], AF.Exp, bias=0.0, scale=1.0
                    )
            else:
                if n % 2 == 0:
                    nc.sync.dma_start(
                        a_t[:], attsT[po].rearrange("(kd p) j c -> p kd j c", p=P)
                    )
                nc.sync.dma_start(
                    x_t[:], images[n].rearrange("(kd p) f -> p kd f", p=P)
                )
                nc.scalar.activation(
                    e_t[:], a_t[:, :, n % 2], AF.Exp, bias=0.0, scale=1.0
                )
            return e_t, x_t

        def emit_e2(n, e_t):
            """Pre-sum E over kd on DVE (f16, 2x rate) so the replicated
            denominator needs only ONE ones-matmul on the PE."""
            e2a = st_pool.tile([P, C], F16, name=f"e2a{n}", tag="e2a")
            nc.vector.tensor_add(e2a[:], e_t[:, 0], e_t[:, 1])
            e2b = st_pool.tile([P, C], F16, name=f"e2b{n}", tag="e2b")
            nc.vector.tensor_add(e2b[:], e_t[:, 2], e_t[:, 3])
            e2 = st_pool.tile([P, C], F16, name=f"e2{n}", tag="e2")
            nc.vector.tensor_add(e2[:], e2a[:], e2b[:])
            return e2

        def emit_ones(n, e2):
            """Replicated denominators: s_ps[p, c] = sum_d E[d, c] (PE)."""
            s_ps = sm_psum.tile([P, C], F32, name=f"s{n}", tag="s", space="PSUM")
            nc.tensor.matmul(s_ps[:], lhsT=ones[:, 0:P], rhs=e2[:])
            s_sb = st_pool.tile([P, C], F16, name=f"ssb{n}", tag="ssb")
            nc.vector.tensor_copy(s_sb[:], s_ps[:])
            return s_sb

        def emit_rp(n, s_sb, r_t):
            """Redistribute s to per-partition layout via tiny PE matmuls,
            then r = 1/s on DVE."""
            rp_ps = sm_psum.tile([P, 2 * KC], F32, name=f"rp{n}", tag="rp", space="PSUM")
            for j in range(KC):
                nc.tensor.matmul(
                    rp_ps[:, j * 2 : (j + 1) * 2],
                    lhsT=s_sb[:, j * P : (j + 1) * P],
                    rhs=oinv[:],
                )
            s_col = st_pool.tile([P, KC], F32, name=f"scol{n}", tag="scol")
            nc.vector.tensor_copy(
                s_col[:],
                rp_ps[:].rearrange("p (kc j) -> p kc j", j=2)[:, :, 0],
            )
            nc.vector.reciprocal(r_t[:], s_col[:])

        def emit_mms(kc, e_t, x_t, ps, ht_major=False):
            if ht_major:
                # ht-major: the first half's accumulation group completes
                # 4 matmuls early, so its eviction overlaps the second
                # half's matmuls (used for the very last band)
                for ht in range(NHT):
                    for kd in range(KD):
                        nc.tensor.matmul(
                            ps[:, ht * NT : (ht + 1) * NT],
                            lhsT=e_t[:, kd, kc * P : (kc + 1) * P],
                            rhs=x_t[:, kd, ht * NT : (ht + 1) * NT],
                            start=(kd == 0),
                            stop=(kd == KD - 1),
                        )
            else:
                for kd in range(KD):
                    lhs = e_t[:, kd, kc * P : (kc + 1) * P]
                    for ht in range(NHT):
                        nc.tensor.matmul(
                            ps[:, ht * NT : (ht + 1) * NT],
                            lhsT=lhs,
                            rhs=x_t[:, kd, ht * NT : (ht + 1) * NT],
                            start=(kd == 0),
                            stop=(kd == KD - 1),
                        )

        def emit_evict(n, kc, ps, r_t, engine, store=None):
            r_ap = r_t[:, kc : kc + 1]
            o_t = o_pool.tile([P, HW], F16, name=f"o{n}_{kc}", tag="o")
            dst = out[n][kc * P : (kc + 1) * P]
            if engine == "vector":
                nc.vector.tensor_scalar_mul(o_t[:], ps[:], r_ap)
            else:
                nc.scalar.mul(o_t[:], ps[:], r_ap)
            if store is None:
                store = nc.gpsimd if kc % 2 == 0 else nc.scalar
            store.dma_start(dst, o_t[:])

        def emit_last_band(n, kc, e_t, x_t, r_t):
            """Final band: two independent 1-bank PSUM tiles (reusing the
            s/rp slots, which are dead by now), so each half's matmul ->
            evict -> store chain is fully independent across engines and
            queues -- minimizes the kernel tail. Stores go on the HWDGE
            rings (scalar + sync), which drain much faster at kernel end
            than the gpsimd SWDGE ring."""
            r_ap = r_t[:, kc : kc + 1]
            dst = out[n][kc * P : (kc + 1) * P]
            ps_a = sm_psum.tile([P, NT], F32, name=f"tla{n}", tag="s", space="PSUM")
            ps_b = sm_psum.tile([P, NT], F32, name=f"tlb{n}", tag="rp", space="PSUM")
            for ht, ps_h in ((0, ps_a), (1, ps_b)):
                for kd in range(KD):
                    nc.tensor.matmul(
                        ps_h[:],
                        lhsT=e_t[:, kd, kc * P : (kc + 1) * P],
                        rhs=x_t[:, kd, ht * NT : (ht + 1) * NT],
                        start=(kd == 0),
                        stop=(kd == KD - 1),
                    )
                if ht == 0:
                    o_a = o_pool.tile([P, NT], F16, name=f"oa{n}", tag="oa")
                    nc.scalar.mul(o_a[:], ps_a[:], r_ap)
                    nc.scalar.dma_start(dst[:, 0:NT], o_a[:])
            o_b = o_pool.tile([P, NT], F16, name=f"ob{n}", tag="ob")
            nc.vector.tensor_scalar_mul(o_b[:], ps_b[:], r_ap)
            # final store split across both idle HWDGE rings: halves the
            # last transfer, whose completion latency gates kernel end
            nc.sync.dma_start(dst[:, NT : NT + NT // 2], o_b[:, 0 : NT // 2])
            nc.scalar.dma_start(dst[:, NT + NT // 2 : HW], o_b[:, NT // 2 : NT])

        # eviction engine per kc: DVE first (its queue is free of prep
        # work), ACT for the later bands (after exp(n+1) has drained)
        EV_ENGINE = ["vector", "vector", "scalar", "scalar"]

        def compute_first(n, e_t, x_t, e2, nxt_e=None):
            """Sample 0 (ramp): kd-outer matmul order over bands 0-2 so each
            arriving (attsT, images) chunk pair immediately feeds 6 matmuls
            -- the PE stays dense (flips the HAM cold window sooner) and
            fully overlaps the chunked loads. Band 3 runs on the two 1-bank
            s/rp PSUM slots so it needn't wait for band 0's eviction."""
            r_t = st_pool.tile([P, KC], F32, name=f"r{n}", tag="r")
            ps = [
                mm_psum.tile([P, HW], F32, name=f"ps{n}_{kc}", tag="ps", space="PSUM")
                for kc in range(3)
            ]
            nxt_e2 = None
            # sample 0 sums the denominator with one ones-matmul per kd
            # group (each needs only its own exp chunk -- no E2 wait).
            # (Alternative with the E2 tree + a single mid-stream ones
            # matmul measured the same within run noise.)
            s_ps = sm_psum.tile([P, C], F32, name=f"s{n}", tag="s", space="PSUM")
            for kd in range(KD):
                for kc in range(3):
                    lhs = e_t[:, kd, kc * P : (kc + 1) * P]
                    for ht in range(NHT):
                        nc.tensor.matmul(
                            ps[kc][:, ht * NT : (ht + 1) * NT],
                            lhsT=lhs,
                            rhs=x_t[:, kd, ht * NT : (ht + 1) * NT],
                            start=(kd == 0),
                            stop=(kd == KD - 1),
                        )
                nc.tensor.matmul(
                    s_ps[:],
                    lhsT=ones[:, 0:P],
                    rhs=e_t[:, kd],
                    start=(kd == 0),
                    stop=(kd == KD - 1),
                )
            s_sb = st_pool.tile([P, C], F16, name=f"ssb{n}", tag="ssb")
            nc.vector.tensor_copy(s_sb[:], s_ps[:])
            emit_rp(n, s_sb, r_t)
            if nxt_e is not None:
                # next sample's E2 adds run in the vector idle window here,
                # before this sample's evictions
                nxt_e2 = emit_e2(n + 1, nxt_e)
            # band 3 on the 1-bank pair (s/rp slots are free by now)
            kc = KC - 1
            dst = out[n][kc * P : (kc + 1) * P]
            r_ap = r_t[:, kc : kc + 1]
            ps3a = sm_psum.tile([P, NT], F32, name=f"f3a{n}", tag="s", space="PSUM")
            ps3b = sm_psum.tile([P, NT], F32, name=f"f3b{n}", tag="rp", space="PSUM")
            for ht, ps_h in ((0, ps3a), (1, ps3b)):
                for kd in range(KD):
                    nc.tensor.matmul(
                        ps_h[:],
                        lhsT=e_t[:, kd, kc * P : (kc + 1) * P],
                        rhs=x_t[:, kd, ht * NT : (ht + 1) * NT],
                        start=(kd == 0),
                        stop=(kd == KD - 1),
                    )
            emit_evict(n, 0, ps[0], r_t, "vector")
            emit_evict(n, 1, ps[1], r_t, "vector")
            emit_evict(n, 2, ps[2], r_t, "scalar")
            o_a = o_pool.tile([P, NT], F16, name=f"oa{n}", tag="oa")
            nc.scalar.mul(o_a[:], ps3a[:], r_ap)
            nc.gpsimd.dma_start(dst[:, 0:NT], o_a[:])
            o_b = o_pool.tile([P, NT], F16, name=f"ob{n}", tag="ob")
            nc.vector.tensor_scalar_mul(o_b[:], ps3b[:], r_ap)
            nc.scalar.dma_start(dst[:, NT:HW], o_b[:])
            return nxt_e2

        def compute(n, e_t, x_t, e2, last=False, nxt_e=None):
            r_t = st_pool.tile([P, KC], F32, name=f"r{n}", tag="r")
            ps_tiles = {}
            s_sb = emit_ones(n, e2)
            ps_tiles[0] = mm_psum.tile([P, HW], F32, name=f"ps{n}_0", tag="ps", space="PSUM")
            emit_mms(0, e_t, x_t, ps_tiles[0])
            emit_rp(n, s_sb, r_t)
            # last sample: no exp(n+1) on scalar, so keep kc0-2 evictions
            # on vector and reserve scalar for the final band's first half;
            # also keep its stores off the slow-draining SWDGE ring
            ev = ["vector", "vector", "vector", "vector"] if last else EV_ENGINE
            st = [nc.sync, nc.scalar, nc.sync, None] if last else [None] * KC
            emit_evict(n, 0, ps_tiles[0], r_t, ev[0], store=st[0])
            nxt_e2 = None
            for kc in range(1, KC):
                if last and kc == KC - 1:
                    emit_last_band(n, kc, e_t, x_t, r_t)
                else:
                    ps = mm_psum.tile([P, HW], F32, name=f"ps{n}_{kc}", tag="ps", space="PSUM")
                    emit_mms(kc, e_t, x_t, ps)
                    emit_evict(n, kc, ps, r_t, ev[kc], store=st[kc])
                if kc == 1 and nxt_e is not None:
                    # deferred: next sample's E2 adds go on the vector queue
                    # AFTER this sample's first evictions so they can never
                    # delay a PSUM slot release
                    nxt_e2 = emit_e2(n + 1, nxt_e)
            return nxt_e2

        # software pipeline: prep one sample ahead so the next sample's
        # exp/loads are never queued behind this sample's evictions
        e0_t, x0_t = prep(0, fine=True)
        e2_cur = None  # sample 0 uses per-kd ones accumulation instead
        staged = (e0_t, x0_t)
        for n in range(NPC):
            nxt = prep(n + 1) if n + 1 < NPC else None
            nxt_e = nxt[0] if nxt is not None else None
            if n == 0:
                e2_cur = compute_first(n, *staged, e2_cur, nxt_e=nxt_e)
            else:
                e2_cur = compute(
                    n, *staged, e2_cur, last=(n == NPC - 1), nxt_e=nxt_e
                )
            staged = nxt

    nc.compile()
    return nc


_NC_CACHE = None


def _get_nc():
    global _NC_CACHE
    if _NC_CACHE is None:
        _NC_CACHE = build_nc()
    return _NC_CACHE


def run(in_maps, **kwargs):
    """Run the SPMD kernel on cores 0..7. in_maps: one dict per core."""
    nc = _get_nc()
    return run_bass_kernel_spmd(nc, in_maps, core_ids=list(range(NCORES)), **kwargs)


def make_in_maps(images: np.ndarray, atts: np.ndarray):
    images = np.ascontiguousarray(
        np.asarray(images, dtype=np.float32).astype(np.float16)
    )
    atts = np.asarray(atts, dtype=np.float32)
    assert images.shape == (N, C, H, W), images.shape
    assert atts.shape == (N, C, C), atts.shape
    img_s = images.reshape(NCORES, NPC, C, HW)
    # pair-packed transpose: attsT[i][po, d, j, c] = atts[i*NPC+2*po+j][c, d]
    a = atts.astype(np.float16).reshape(NCORES, NPC // 2, 2, C, C)
    attsT = np.ascontiguousarray(a.transpose(0, 1, 4, 2, 3))
    return [
        {"images": np.ascontiguousarray(img_s[i]), "attsT": attsT[i]}
        for i in range(NCORES)
    ]


def kernel(images: np.ndarray, atts: np.ndarray) -> np.ndarray:
    in_maps = make_in_maps(images, atts)
    res = run(in_maps)
    outs = [res.results[i]["out"] for i in range(NCORES)]
    full = np.concatenate(outs, axis=0).reshape(N, C, H, W)
    return full.astype(np.float32)
